# revision 1
# baseline (speedup 1.0000x reference)
"""ConfidenceBiasedCrossAttention Trainium2 kernel (8 NeuronCores).

Sharding (Megatron-style, per spec hint): data-parallel over B (2) x
head-parallel over 4 head-groups of 4 heads (256 channels) -> 8 cores.
Each core computes q/k/v projections for its 256 channels, biased
softmax attention for its 4 heads, and a partial output projection
(rows of Wo). Host sums the 4 partials per batch and adds the bias.

The V-projection bias is folded into the host-side bias: softmax
weights sum to 1, so attn @ (v + bv) = attn @ v + bv, and
sum_g bv_g @ Wo_g.T = Wv_b @ Wo_w.T is added on the host (exact).

Device pipeline per core (all matmuls in fp32r = fast fp32):
  - PE-transpose Q/K/V blocks and weight slices (f32r transpose mode)
  - qT/kT [256ch, L] and v [Lk, 256ch(+ones col)] projections
  - logitsT [Lk-chunk, Lq] = k_h qT_h; exp(0.125*x + V_bias[k]) on ACT
  - [attn_out.T | denom] accumulated over Lk chunks in PSUM via the
    ones column; normalize with reciprocal + K=1 broadcast matmul
  - partial out = attnT.T @ WoT accumulated over the 256 channels

PSUM budget (8 banks): pmm [128,256]x2 (transposes + proj matmuls,
1 bank each) + lg [128,1024]x2 (logits/bcast/Wo, 2 banks each) +
acc [128,1024]x1 (attn accumulator, 2 banks) = 8. Separate tags keep
the attention stream decoupled from the projection pipeline so the
scheduler can overlap them.
"""

import numpy as np

import concourse.bacc as bacc
import concourse.mybir as mybir
import concourse.tile as tile
from concourse import bass_utils
from concourse.masks import make_identity

F32 = mybir.dt.float32
F32R = mybir.dt.float32r
AF = mybir.ActivationFunctionType
MUL = mybir.AluOpType.mult

P = 128
C = 1024
D = 64
LQ = 1024
LK = 4096
CS = 256          # channels per core (4 heads)
NH = 4            # heads per core
SCALE = 1.0 / 8.0
BLK = 256         # Lk/Lq rows per processing block
NKB = LK // BLK   # 16
NQB = LQ // BLK   # 4
NCH = LK // P     # 32 Lk chunks of 128

TRANSPOSE_F32R = True  # f32r transpose mode: 1.5 cyc/row vs 2.0 for fp32


def _transpose_block(nc, ps, ident, dst, srcs):
    """Transpose [128,128] blocks of src (natural [row, ch]) into dst
    [128(ch-chunk), len*128(rows)] via PSUM."""
    pt = ps.tile([P, len(srcs) * P], F32R if TRANSPOSE_F32R else F32,
                 tag="pmm", bufs=2)
    for a, s in enumerate(srcs):
        nc.tensor.transpose(pt[:, a * P : (a + 1) * P], s, ident)
    nc.vector.tensor_copy(dst, pt if TRANSPOSE_F32R else pt)


def build_nc():
    nc = bacc.Bacc("TRN2", target_bir_lowering=False, debug=False, num_devices=8)
    XDT = F32R if TRANSPOSE_F32R else F32
    qb_d = nc.dram_tensor("Qb", [LQ, C], F32, kind="ExternalInput").ap()
    kb_d = nc.dram_tensor("Kb", [LK, C], F32, kind="ExternalInput").ap()
    vb_d = nc.dram_tensor("Vb", [LK, C], F32, kind="ExternalInput").ap()
    vbias_d = nc.dram_tensor("vbias", [P, NCH], F32, kind="ExternalInput").ap()
    wq_d = nc.dram_tensor("wq", [CS, C], F32, kind="ExternalInput").ap()
    wk_d = nc.dram_tensor("wk", [CS, C], F32, kind="ExternalInput").ap()
    wv_d = nc.dram_tensor("wv", [CS, C], F32, kind="ExternalInput").ap()
    wo_d = nc.dram_tensor("wo", [C, CS], F32, kind="ExternalInput").ap()
    bq_d = nc.dram_tensor("bq", [P, 2], F32, kind="ExternalInput").ap()
    bk_d = nc.dram_tensor("bk", [P, 2], F32, kind="ExternalInput").ap()
    out_d = nc.dram_tensor("out", [LQ, C], F32, kind="ExternalOutput").ap()

    def dram_x(ap):
        return ap.bitcast(F32R) if TRANSPOSE_F32R else ap

    with tile.TileContext(nc) as tc:
        with (
            tc.tile_pool(name="pers", bufs=1) as pers,
            tc.tile_pool(name="sb", bufs=1) as sb,
            tc.tile_pool(name="ps", bufs=2, space="PSUM") as ps,
        ):
            # ---- constants ----
            ident_f = pers.tile([P, P], F32)
            make_identity(nc, ident_f)
            if TRANSPOSE_F32R:
                ident = pers.tile([P, P], F32R)
                nc.vector.tensor_copy(ident, ident_f)
            else:
                ident = ident_f
            ones_f32 = pers.tile([P, 1], F32)
            nc.gpsimd.memset(ones_f32, 1.0)
            ones_r = pers.tile([1, P], F32R)
            nc.vector.tensor_copy(ones_r, ones_f32[0:1, :].to_broadcast([1, P]))
            vbias_sb = pers.tile([P, NCH], F32)
            nc.sync.dma_start(vbias_sb, vbias_d)
            bq_sb = pers.tile([P, 2], F32)
            nc.sync.dma_start(bq_sb, bq_d)
            bk_sb = pers.tile([P, 2], F32)
            nc.sync.dma_start(bk_sb, bk_d)

            # ---- persistent activations ----
            qT = pers.tile([P, 2, LQ], F32R)       # [ch%128, ch//128, Lq]
            kT = pers.tile([P, 2, LK], F32R)
            v65 = pers.tile([P, NCH, NH, D + 1], F32R)  # [k%128, chunk, h, v|1]
            attnT = pers.tile([P, 2, LQ], F32R)
            wqT = pers.tile([P, 8, CS], F32R)      # [cin%128, cin//128, cout]
            wkT = pers.tile([P, 8, CS], F32R)
            wvT = pers.tile([P, 8, CS], F32R)
            woT = pers.tile([P, 2, C], F32R)       # [ch%128, ch//128, cout]

            # ones column of v65 (denominator trick)
            nc.vector.tensor_copy(
                v65[:, :, :, D].rearrange("p a b -> p (a b)"),
                ones_f32.to_broadcast([P, NCH * NH]),
            )

            # ---- weight transposes ----
            for w_d_, wT in ((wq_d, wqT), (wk_d, wkT), (wv_d, wvT)):
                w_nat = sb.tile([P, 2, C], XDT, tag="wnat", bufs=2)
                nc.sync.dma_start(w_nat, dram_x(w_d_).rearrange("(t p) c -> p t c", p=P))
                for i in range(8):
                    _transpose_block(
                        nc, ps, ident, wT[:, i, :],
                        [w_nat[:, mt, i * P : (i + 1) * P] for mt in range(2)],
                    )
            wo_nat = sb.tile([P, 8, CS], XDT, tag="wnat", bufs=2)
            nc.sync.dma_start(wo_nat, dram_x(wo_d).rearrange("(t p) c -> p t c", p=P))
            for kc in range(2):
                pw = ps.tile([P, C], F32R if TRANSPOSE_F32R else F32, tag="lg")
                for j in range(8):
                    nc.tensor.transpose(
                        pw[:, j * P : (j + 1) * P],
                        wo_nat[:, j, kc * P : (kc + 1) * P],
                        ident,
                    )
                nc.vector.tensor_copy(woT[:, kc, :], pw)

            # ---- Q projection (4 blocks of 256 rows) ----
            def proj_block(x_d, blk, outs):
                xin = sb.tile([P, 2, C], XDT, tag="xin", bufs=2)
                nc.sync.dma_start(
                    xin,
                    dram_x(x_d[blk * BLK : (blk + 1) * BLK, :]).rearrange(
                        "(t p) c -> p t c", p=P
                    ),
                )
                xt = sb.tile([P, 8, BLK], F32R, tag="xt", bufs=2)
                for i in range(8):
                    _transpose_block(
                        nc, ps, ident, xt[:, i, :],
                        [xin[:, a, i * P : (i + 1) * P] for a in range(2)],
                    )
                outs(xt)

            def qk_out(wT, bias_sb, dstT, blk):
                def _o(xt):
                    for mt in range(2):
                        pq = ps.tile([P, BLK], F32, tag="pmm", bufs=2)
                        for i in range(8):
                            nc.tensor.matmul(
                                pq, wT[:, i, mt * P : (mt + 1) * P], xt[:, i, :],
                                start=(i == 0), stop=(i == 7),
                            )
                        nc.vector.tensor_scalar_add(
                            dstT[:, mt, blk * BLK : (blk + 1) * BLK], pq,
                            bias_sb[:, mt : mt + 1],
                        )
                return _o

            for blk in range(NQB):
                proj_block(qb_d, blk, qk_out(wqT, bq_sb, qT, blk))

            # ---- K & V projections, interleaved blocks of 256 rows ----
            def v_out(blk):
                def _o(xt):
                    for a in range(2):
                        pv = ps.tile([P, CS], F32, tag="pmm", bufs=2)
                        for i in range(8):
                            nc.tensor.matmul(
                                pv, xt[:, i, a * P : (a + 1) * P], wvT[:, i, :],
                                start=(i == 0), stop=(i == 7),
                            )
                        nc.vector.tensor_copy(
                            v65[:, blk * 2 + a, :, 0:D],
                            pv.rearrange("p (h d) -> p h d", d=D),
                        )
                return _o

            # ---- attention helpers ----
            def attn_chunk(h, c, po):
                ht, hp = h // 2, (h % 2) * D
                pl = ps.tile([P, LQ], F32, tag="lg", bufs=2, name="pl")
                for n in range(2):
                    nc.tensor.matmul(
                        pl[:, n * 512 : (n + 1) * 512],
                        kT[hp : hp + D, ht, c * P : (c + 1) * P],
                        qT[hp : hp + D, ht, n * 512 : (n + 1) * 512],
                        start=True, stop=True, tile_position=(hp, 0),
                    )
                eT = sb.tile([P, LQ], F32R, tag="exp", bufs=3, name="eT")
                nc.scalar.activation(
                    eT, pl, AF.Exp, bias=vbias_sb[:, c : c + 1], scale=SCALE
                )
                for n in range(2):
                    nc.tensor.matmul(
                        po[0 : D + 1, n * 512 : (n + 1) * 512],
                        v65[:, c, h, :],
                        eT[:, n * 512 : (n + 1) * 512],
                        start=(c == 0), stop=(c == NCH - 1),
                    )

            def attn_finish(h, po):
                ht, hp = h // 2, (h % 2) * D
                rec = sb.tile([1, LQ], F32R, tag="rec", name="rec")
                with nc.allow_low_precision(reason="softmax denom reciprocal"):
                    nc.vector.reciprocal(rec, po[D : D + 1, :])
                pb = ps.tile([P, LQ], F32, tag="lg", bufs=2, name="pb")
                for n in range(2):
                    nc.tensor.matmul(
                        pb[0:D, n * 512 : (n + 1) * 512],
                        ones_r[:, 0:D],
                        rec[:, n * 512 : (n + 1) * 512],
                        start=True, stop=True,
                    )
                bc = sb.tile([D, LQ], F32, tag="bc", name="bc")
                nc.vector.tensor_copy(bc, pb[0:D, :])
                nc.vector.tensor_tensor(
                    attnT[hp : hp + D, ht, :], po[0:D, :], bc, MUL
                )

            # head 0 streams behind the K/V projection blocks (its chunk c
            # only needs block c//2); heads 1-3 run as the tail.
            po0 = ps.tile([P, LQ], F32, tag="acc", bufs=1, name="po0")
            for blk in range(NKB):
                proj_block(kb_d, blk, qk_out(wkT, bk_sb, kT, blk))
                proj_block(vb_d, blk, v_out(blk))
                attn_chunk(0, 2 * blk, po0)
                attn_chunk(0, 2 * blk + 1, po0)
            attn_finish(0, po0)
            for h in range(1, NH):
                po = ps.tile([P, LQ], F32, tag="acc", bufs=1, name="po")
                for c in range(NCH):
                    attn_chunk(h, c, po)
                attn_finish(h, po)

            # ---- output projection (partial; host adds bias + reduces) ----
            for m in range(8):
                pw = ps.tile([P, C], F32, tag="lg", bufs=2)
                for kc in range(2):
                    for n in range(2):
                        nc.tensor.matmul(
                            pw[:, n * 512 : (n + 1) * 512],
                            attnT[:, kc, m * P : (m + 1) * P],
                            woT[:, kc, n * 512 : (n + 1) * 512],
                            start=(kc == 0), stop=(kc == 1),
                        )
                ob = sb.tile([P, C], F32, tag="ob", bufs=2)
                nc.vector.tensor_copy(ob, pw)
                nc.sync.dma_start(out_d[m * P : (m + 1) * P, :], ob)

    nc.compile()
    return nc


_NC = None


def _get_nc():
    global _NC
    if _NC is None:
        _NC = build_nc()
    return _NC


def shard_inputs(Q, K_in, V_in, V_bias, Wq_w, Wq_b, Wk_w, Wk_b, Wv_w, Wv_b, Wo_w, Wo_b):
    """Build the 8 per-core input dicts."""
    in_maps = []
    for core in range(8):
        b, g = core // 4, core % 4
        gs, ge = g * CS, (g + 1) * CS
        in_maps.append({
            "Qb": np.ascontiguousarray(Q[b]),
            "Kb": np.ascontiguousarray(K_in[b]),
            "Vb": np.ascontiguousarray(V_in[b]),
            "vbias": np.ascontiguousarray(V_bias[b].reshape(NCH, P).T),
            "wq": np.ascontiguousarray(Wq_w[gs:ge]),
            "wk": np.ascontiguousarray(Wk_w[gs:ge]),
            "wv": np.ascontiguousarray(Wv_w[gs:ge]),
            "wo": np.ascontiguousarray(Wo_w[:, gs:ge]),
            "bq": np.ascontiguousarray(Wq_b[gs:ge].reshape(2, P).T),
            "bk": np.ascontiguousarray(Wk_b[gs:ge].reshape(2, P).T),
        })
    return in_maps


def combine_outputs(results, Wv_b, Wo_w, Wo_b):
    """Sum the 4 head-group partials per batch; add output bias and the
    folded V-projection bias (attention weights sum to 1)."""
    bias = Wo_b + Wv_b @ Wo_w.T
    outs = np.stack([r["out"] for r in results]).reshape(2, 4, LQ, C)
    return (outs.sum(axis=1) + bias[None, None, :]).astype(np.float32)


def kernel(**inputs):
    nc = _get_nc()
    in_maps = shard_inputs(**inputs)
    res = bass_utils.run_bass_kernel_spmd(nc, in_maps, core_ids=list(range(8)))
    return combine_outputs(
        res.results,
        np.asarray(inputs["Wv_b"]),
        np.asarray(inputs["Wo_w"]),
        np.asarray(inputs["Wo_b"]),
    )


if __name__ == "__main__":
    rng = np.random.default_rng(0)
    ins = {
        "Q": rng.standard_normal((2, LQ, C), dtype=np.float32),
        "K_in": rng.standard_normal((2, LK, C), dtype=np.float32),
        "V_in": rng.standard_normal((2, LK, C), dtype=np.float32),
        "V_bias": rng.standard_normal((2, LK)).astype(np.float32),
        **{
            f"W{x}_w": (rng.standard_normal((C, C)) * 0.03).astype(np.float32)
            for x in "qkvo"
        },
        **{
            f"W{x}_b": (rng.standard_normal(C) * 0.03).astype(np.float32)
            for x in "qkvo"
        },
    }
    out = kernel(**ins)
    print("ok", out.shape, out.dtype)



# revision 5
# speedup vs baseline: 1.6033x; 1.6033x over previous
"""ConfidenceBiasedCrossAttention Trainium2 kernel (8 NeuronCores), v3.

Sharding (Megatron-style): data-parallel over B (2) x head-parallel over
4 head-groups of 4 heads (256 channels) -> 8 cores. Each core computes
q/k/v projections for its 256 channels, biased softmax attention for its
4 heads, and a partial output projection (rows of Wo). Host sums the 4
partials per batch and adds the bias.

v2: host-side transposes + bf16 casts (no device transposes, half DMA).
v3 overlap fixes on top:
  - DMAs spread across queues: SP carries weights/biases + output, DVE
    carries Q, Pool streams the 16 K/V half-tiles. Tiles split in halves
    so compute starts after ~3us, not ~16us.
  - Each head's softmax-finish (reciprocal -> broadcast matmul -> scale)
    is interleaved into the NEXT head's chunk stream with a 3-chunk AV
    lag, so the in-order PE queue never stalls on the DVE chain.
  - Output tail pipelines PSUM->SBUF copies on alternating DVE/ACT and
    DMAs on alternating SP/Pool queues; the last tile is split in half.

The V-projection bias is folded into the host-side bias: softmax weights
sum to 1, so attn @ (v + bv) = attn @ v + bv, and sum_g bv_g @ Wo_g.T =
Wv_b @ Wo_w.T is added on the host (exact).

PSUM budget (8 banks): pp [128,512]x2 (projections + bcast, 1 bank each)
+ lg [128,1024]x2 (logits/outproj, 2 banks each) + acc [128,1024]x1
(attn accumulator, 2 banks) = 8.
"""

import numpy as np
import ml_dtypes

import concourse.bacc as bacc
import concourse.mybir as mybir
import concourse.tile as tile
from concourse import bass_utils

F32 = mybir.dt.float32
F32R = mybir.dt.float32r
BF16 = mybir.dt.bfloat16
NPBF16 = ml_dtypes.bfloat16
AF = mybir.ActivationFunctionType
MUL = mybir.AluOpType.mult

P = 128
C = 1024
D = 64
LQ = 1024
LK = 4096
CS = 256          # channels per core (4 heads)
NH = 4            # heads per core
SCALE = 1.0 / 8.0
NCH = LK // P     # 32 key chunks of 128
NQT = 4           # key quarters streamed from HBM
QK = LK // NQT    # 1024 keys per quarter
LAG = 3           # AV matmul lag behind logits in a head's chunk stream


def build_nc():
    nc = bacc.Bacc("TRN2", target_bir_lowering=False, debug=False, num_devices=8)
    qt_d = nc.dram_tensor("qt", [C, LQ], BF16, kind="ExternalInput").ap()
    kt_d = nc.dram_tensor("kt", [C, LK], BF16, kind="ExternalInput").ap()
    vt_d = nc.dram_tensor("vt", [C, LK], BF16, kind="ExternalInput").ap()
    wq_d = nc.dram_tensor("wq", [C, CS], BF16, kind="ExternalInput").ap()
    wk_d = nc.dram_tensor("wk", [C, CS], BF16, kind="ExternalInput").ap()
    wv_d = nc.dram_tensor("wv", [C, CS], BF16, kind="ExternalInput").ap()
    wo_d = nc.dram_tensor("wo", [CS, C], BF16, kind="ExternalInput").ap()
    vbias_d = nc.dram_tensor("vbias", [P, NCH], F32, kind="ExternalInput").ap()
    bq_d = nc.dram_tensor("bq", [P, 2], F32, kind="ExternalInput").ap()
    bk_d = nc.dram_tensor("bk", [P, 2], F32, kind="ExternalInput").ap()
    out_d = nc.dram_tensor("out", [LQ, C], F32, kind="ExternalOutput").ap()

    with tile.TileContext(nc) as tc:
        with (
            tc.tile_pool(name="pers", bufs=1) as pers,
            tc.tile_pool(name="sb", bufs=1) as sb,
            tc.tile_pool(name="ps", bufs=2, space="PSUM") as ps,
        ):
            # ---- weights (host-transposed: [c_in, c_out]); SP queue ----
            wqs = pers.tile([P, 8, CS], BF16)
            wks = pers.tile([P, 8, CS], BF16)
            wvs = pers.tile([P, 8, CS], BF16)
            wos = pers.tile([P, 2, C], BF16)
            nc.sync.dma_start(wqs, wq_d.rearrange("(t p) o -> p t o", p=P))
            nc.sync.dma_start(wks, wk_d.rearrange("(t p) o -> p t o", p=P))
            bq_sb = pers.tile([P, 2], F32)
            nc.sync.dma_start(bq_sb, bq_d)
            bk_sb = pers.tile([P, 2], F32)
            nc.sync.dma_start(bk_sb, bk_d)
            vbias_sb = pers.tile([P, NCH], F32)
            nc.sync.dma_start(vbias_sb, vbias_d)
            nc.sync.dma_start(wvs, wv_d.rearrange("(t p) o -> p t o", p=P))
            nc.sync.dma_start(wos, wo_d.rearrange("(t p) o -> p t o", p=P))

            # ---- Q staging in halves; DVE queue ----
            qtc = []
            for n in range(2):
                t = sb.tile([P, 8, 512], BF16, tag="qstage", bufs=2, name="qtc")
                nc.scalar.dma_start(
                    t, qt_d[:, n * 512 : (n + 1) * 512].rearrange("(t p) r -> p t r", p=P)
                )
                qtc.append(t)

            # ---- constants ----
            ones_f32 = pers.tile([P, 1], F32)
            nc.gpsimd.memset(ones_f32, 1.0)
            ones_bf = pers.tile([P, 1], BF16)
            nc.vector.tensor_copy(ones_bf, ones_f32)
            ones_r = pers.tile([1, P], F32R)
            nc.vector.tensor_copy(ones_r, ones_f32[0:1, :].to_broadcast([1, P]))

            # ---- persistent activations ----
            qT = pers.tile([P, 2, LQ], BF16)        # [ch%128, ch//128, q]
            kT = pers.tile([P, 2, LK], BF16)        # [ch%128, ch//128, key]
            v65 = pers.tile([P, NCH, NH, D + 1], BF16)  # [key%128, chunk, h, v|1]
            attnT = pers.tile([P, 2, LQ], BF16)

            # ones column of v65 (denominator trick)
            nc.vector.tensor_copy(
                v65[:, :, :, D].rearrange("p a b -> p (a b)"),
                ones_bf.to_broadcast([P, NCH * NH]),
            )

            # ---- Q projection ----
            for n in range(2):
                for mt in range(2):
                    pq = ps.tile([P, 512], F32, tag="pp", bufs=2, name="pq")
                    for i in range(8):
                        nc.tensor.matmul(
                            pq, wqs[:, i, mt * P : (mt + 1) * P], qtc[n][:, i, :],
                            start=(i == 0), stop=(i == 7),
                        )
                    nc.vector.tensor_scalar_add(
                        qT[:, mt, n * 512 : (n + 1) * 512], pq, bq_sb[:, mt : mt + 1]
                    )

            # ---- K & V projection for one key-quarter; Pool DMA queue ----
            def proj_quarter(qb):
                for n in range(2):
                    ktq = sb.tile([P, 8, 512], BF16, tag="kstage", bufs=2, name="ktq")
                    lo = qb * QK + n * 512
                    nc.gpsimd.dma_start(
                        ktq, kt_d[:, lo : lo + 512].rearrange("(t p) k -> p t k", p=P)
                    )
                    for mt in range(2):
                        pk = ps.tile([P, 512], F32, tag="pp", bufs=2, name="pk")
                        for i in range(8):
                            nc.tensor.matmul(
                                pk, wks[:, i, mt * P : (mt + 1) * P], ktq[:, i, :],
                                start=(i == 0), stop=(i == 7),
                            )
                        nc.vector.tensor_scalar_add(
                            kT[:, mt, lo : lo + 512], pk, bk_sb[:, mt : mt + 1]
                        )
                for n in range(2):
                    vtq = sb.tile([P, 8, 512], BF16, tag="vstage", bufs=2, name="vtq")
                    lo = qb * QK + n * 512
                    nc.gpsimd.dma_start(
                        vtq, vt_d[:, lo : lo + 512].rearrange("(t p) k -> p t k", p=P)
                    )
                    for a in range(4):
                        pv = ps.tile([P, 512], F32, tag="pp", bufs=2, name="pv")
                        for i in range(8):
                            nc.tensor.matmul(
                                pv[:, 0:CS], vtq[:, i, a * P : (a + 1) * P], wvs[:, i, :],
                                start=(i == 0), stop=(i == 7),
                            )
                        nc.vector.tensor_copy(
                            v65[:, qb * 8 + n * 4 + a, :, 0:D],
                            pv[:, 0:CS].rearrange("p (h d) -> p h d", d=D),
                        )

            # ---- attention pieces ----
            e_tiles = {}

            def logits_exp(h, c):
                ht, hp = h // 2, (h % 2) * D
                pl = ps.tile([P, LQ], F32, tag="lg", bufs=2, name="pl")
                for n in range(2):
                    nc.tensor.matmul(
                        pl[:, n * 512 : (n + 1) * 512],
                        kT[hp : hp + D, ht, c * P : (c + 1) * P],
                        qT[hp : hp + D, ht, n * 512 : (n + 1) * 512],
                        start=True, stop=True, tile_position=(hp, 0),
                    )
                eT = sb.tile([P, LQ], BF16, tag="exp", bufs=4, name="eT")
                nc.scalar.activation(
                    eT, pl, AF.Exp, bias=vbias_sb[:, c : c + 1], scale=SCALE
                )
                e_tiles[(h, c)] = eT

            def av(h, c, po):
                eT = e_tiles.pop((h, c))
                for n in range(2):
                    nc.tensor.matmul(
                        po[0 : D + 1, n * 512 : (n + 1) * 512],
                        v65[:, c, h, :],
                        eT[:, n * 512 : (n + 1) * 512],
                        start=(c == 0), stop=(c == NCH - 1),
                    )

            def make_finish(h, po):
                """Finish pieces for head h, to be interleaved into the next
                head's stream: rec (DVE), pb+bc (PE+DVE), tt (DVE)."""
                ht, hp = h // 2, (h % 2) * D
                state = {}

                def rec():
                    r = sb.tile([1, LQ], F32R, tag="rec", bufs=2, name="rec")
                    with nc.allow_low_precision(reason="softmax denom reciprocal"):
                        nc.vector.reciprocal(r, po[D : D + 1, :])
                    state["rec"] = r

                def pb_bc():
                    bc = sb.tile([D, LQ], F32, tag="bc", bufs=2, name="bc")
                    for n in range(2):
                        pb = ps.tile([P, 512], F32, tag="pp", bufs=2, name="pb")
                        nc.tensor.matmul(
                            pb[0:D, :], ones_r[:, 0:D],
                            state["rec"][:, n * 512 : (n + 1) * 512],
                            start=True, stop=True,
                        )
                        nc.vector.tensor_copy(bc[:, n * 512 : (n + 1) * 512], pb[0:D, :])
                    state["bc"] = bc

                def tt():
                    nc.vector.tensor_tensor(
                        attnT[hp : hp + D, ht, :], po[0:D, :], state["bc"], MUL
                    )

                return (rec, pb_bc, tt)

            # head 0 streams behind the K/V projection quarters (chunk c only
            # needs quarter c//8); heads 1-3 run as the tail with the previous
            # head's finish interleaved into the first chunks.
            po0 = ps.tile([P, LQ], F32, tag="acc", bufs=1, name="po0")
            for qb in range(NQT):
                proj_quarter(qb)
                for cc in range(8):
                    c = qb * 8 + cc
                    logits_exp(0, c)
                    av(0, c, po0)
            fin = make_finish(0, po0)

            for h in range(1, NH):
                po = ps.tile([P, LQ], F32, tag="acc", bufs=1, name="po")
                for c in range(NCH):
                    logits_exp(h, c)
                    if c < len(fin):
                        fin[c]()
                    if c >= LAG:
                        av(h, c - LAG, po)
                for c in range(NCH - LAG, NCH):
                    av(h, c, po)
                fin = make_finish(h, po)

            # final head's finish, then output projection
            for piece in fin:
                piece()

            # ---- output projection (partial; host adds bias + reduces) ----
            for m in range(8):
                pw = ps.tile([P, C], F32, tag="lg", bufs=2, name="pw")
                for n in range(2):
                    for kc in range(2):
                        nc.tensor.matmul(
                            pw[:, n * 512 : (n + 1) * 512],
                            attnT[:, kc, m * P : (m + 1) * P],
                            wos[:, kc, n * 512 : (n + 1) * 512],
                            start=(kc == 0), stop=(kc == 1),
                        )
                ob = sb.tile([P, C], F32, tag="ob", bufs=3, name="ob")
                nc.vector.tensor_copy(ob[:, 0:512], pw[:, 0:512])
                nc.scalar.activation(ob[:, 512:1024], pw[:, 512:1024], AF.Copy)
                nc.sync.dma_start(out_d[m * P : (m + 1) * P, 0:512], ob[:, 0:512])
                nc.gpsimd.dma_start(
                    out_d[m * P : (m + 1) * P, 512:1024], ob[:, 512:1024]
                )

    nc.compile()
    return nc


_NC = None


def _get_nc():
    global _NC
    if _NC is None:
        _NC = build_nc()
    return _NC


def shard_inputs(Q, K_in, V_in, V_bias, Wq_w, Wq_b, Wk_w, Wk_b, Wv_w, Wv_b, Wo_w, Wo_b):
    """Build the 8 per-core input dicts (host transposes + bf16 casts)."""
    Q = np.asarray(Q)
    K_in = np.asarray(K_in)
    V_in = np.asarray(V_in)
    V_bias = np.asarray(V_bias)
    per_batch = []
    for b in range(2):
        per_batch.append({
            "qt": np.ascontiguousarray(Q[b].T).astype(NPBF16),
            "kt": np.ascontiguousarray(K_in[b].T).astype(NPBF16),
            "vt": np.ascontiguousarray(V_in[b].T).astype(NPBF16),
            "vbias": np.ascontiguousarray(V_bias[b].reshape(NCH, P).T),
        })
    in_maps = []
    for core in range(8):
        b, g = core // 4, core % 4
        gs, ge = g * CS, (g + 1) * CS
        in_maps.append({
            **per_batch[b],
            "wq": np.ascontiguousarray(Wq_w[gs:ge].T).astype(NPBF16),
            "wk": np.ascontiguousarray(Wk_w[gs:ge].T).astype(NPBF16),
            "wv": np.ascontiguousarray(Wv_w[gs:ge].T).astype(NPBF16),
            "wo": np.ascontiguousarray(Wo_w[:, gs:ge].T).astype(NPBF16),
            "bq": np.ascontiguousarray(Wq_b[gs:ge].reshape(2, P).T),
            "bk": np.ascontiguousarray(Wk_b[gs:ge].reshape(2, P).T),
        })
    return in_maps


def combine_outputs(results, Wv_b, Wo_w, Wo_b):
    """Sum the 4 head-group partials per batch; add output bias and the
    folded V-projection bias (attention weights sum to 1)."""
    bias = Wo_b + Wv_b @ Wo_w.T
    outs = np.stack([r["out"] for r in results]).reshape(2, 4, LQ, C)
    return (outs.sum(axis=1) + bias[None, None, :]).astype(np.float32)


def kernel(**inputs):
    nc = _get_nc()
    in_maps = shard_inputs(**inputs)
    res = bass_utils.run_bass_kernel_spmd(nc, in_maps, core_ids=list(range(8)))
    return combine_outputs(
        res.results,
        np.asarray(inputs["Wv_b"]),
        np.asarray(inputs["Wo_w"]),
        np.asarray(inputs["Wo_b"]),
    )


if __name__ == "__main__":
    rng = np.random.default_rng(0)
    ins = {
        "Q": rng.standard_normal((2, LQ, C), dtype=np.float32),
        "K_in": rng.standard_normal((2, LK, C), dtype=np.float32),
        "V_in": rng.standard_normal((2, LK, C), dtype=np.float32),
        "V_bias": rng.standard_normal((2, LK)).astype(np.float32),
        **{
            f"W{x}_w": (rng.standard_normal((C, C)) * 0.03).astype(np.float32)
            for x in "qkvo"
        },
        **{
            f"W{x}_b": (rng.standard_normal(C) * 0.03).astype(np.float32)
            for x in "qkvo"
        },
    }
    out = kernel(**ins)
    print("ok", out.shape, out.dtype)


# revision 27
# speedup vs baseline: 4.4835x; 2.7965x over previous
"""ConfidenceBiasedCrossAttention Trainium2 kernel (8 NeuronCores), v5.

Sharding (Megatron-style): data-parallel over B (2) x head-parallel over
4 head-groups of 4 heads (256 channels) -> 8 cores. Each core computes
q/k/v projections for its 256 channels, biased softmax attention for its
4 heads, and a partial output projection (rows of Wo). Host sums the 4
partials per batch and adds the bias.

v2: host-side transposes + bf16 casts (no device transposes, half DMA).
v3/v4: multi-queue DMA, interleaved softmax-finish, pipelined tail.
v5: phase-balance PE vs ACT. The exp stream (128 x [128,1024] tiles,
~1.04us each on ACT) exceeds PE matmul work during the attention tail,
so heads are rescheduled:
  - Phase 1 (PE-bound): projections + heads 0 AND 1 fully streamed (two
    PSUM accumulators) + head 2's logits+exp computed and STORED in SBUF
    (e2T, 64KB/part). 96 of 128 exps hide under ~129us of PE work.
  - Phase 2 (PE-bound): head 3 logits/exp live, head 2's AVs replayed
    from e2T (no ACT cost), head 3 AVs lagged. ACT has only 33us left.
  - K/V/Q projection PSUM groups borrow the pl tag, so PSUM is exactly
    8 banks: pl [128,1024]x2 + acc [128,1024]x2.

The V-projection bias is folded into the host-side bias: softmax weights
sum to 1, so attn @ (v + bv) = attn @ v + bv, and sum_g bv_g @ Wo_g.T =
Wv_b @ Wo_w.T is added on the host (exact).
"""

import numpy as np
import ml_dtypes

import concourse.bacc as bacc
import concourse.mybir as mybir
import concourse.tile as tile
from concourse import bass_utils

F32 = mybir.dt.float32
F32R = mybir.dt.float32r
BF16 = mybir.dt.bfloat16
NPBF16 = ml_dtypes.bfloat16
AF = mybir.ActivationFunctionType
MUL = mybir.AluOpType.mult

P = 128
C = 1024
D = 64
LQ = 1024
LK = 4096
CS = 256          # channels per core (4 heads)
NH = 4            # heads per core
SCALE = 1.0 / 8.0
NCH = LK // P     # 32 key chunks of 128
NQT = 4           # key quarters streamed from HBM
QK = LK // NQT    # 1024 keys per quarter


def build_nc():
    nc = bacc.Bacc("TRN2", target_bir_lowering=False, debug=False, num_devices=8)
    qt_d = nc.dram_tensor("qt", [C, LQ], BF16, kind="ExternalInput").ap()
    kt_d = nc.dram_tensor("kt", [C, LK], BF16, kind="ExternalInput").ap()
    vt_d = nc.dram_tensor("vt", [C, LK], BF16, kind="ExternalInput").ap()
    wq_d = nc.dram_tensor("wq", [C, CS], BF16, kind="ExternalInput").ap()
    wk_d = nc.dram_tensor("wk", [C, CS], BF16, kind="ExternalInput").ap()
    wv_d = nc.dram_tensor("wv", [C, CS], BF16, kind="ExternalInput").ap()
    wo_d = nc.dram_tensor("wo", [CS, C], BF16, kind="ExternalInput").ap()
    vbias_d = nc.dram_tensor("vbias", [P, NCH], F32, kind="ExternalInput").ap()
    bq_d = nc.dram_tensor("bq", [P, 2], F32, kind="ExternalInput").ap()
    bk_d = nc.dram_tensor("bk", [P, 2], F32, kind="ExternalInput").ap()
    out_d = nc.dram_tensor("out", [LQ, C], F32, kind="ExternalOutput").ap()

    with tile.TileContext(nc) as tc:
        with (
            tc.tile_pool(name="pers", bufs=1) as pers,
            tc.tile_pool(name="sb", bufs=1) as sb,
            tc.tile_pool(name="ps", bufs=2, space="PSUM") as ps,
        ):
            # ---- weights (host-transposed: [c_in, c_out]); SP queue ----
            wqs = pers.tile([P, 8, CS], BF16)
            wks = pers.tile([P, 8, CS], BF16)
            wvs = pers.tile([P, 8, CS], BF16)
            wos = pers.tile([P, 2, C], BF16)
            nc.sync.dma_start(wqs, wq_d.rearrange("(t p) o -> p t o", p=P))
            bq_sb = pers.tile([P, 2], F32)
            nc.sync.dma_start(bq_sb, bq_d)
            nc.sync.dma_start(wks, wk_d.rearrange("(t p) o -> p t o", p=P))
            bk_sb = pers.tile([P, 2], F32)
            nc.sync.dma_start(bk_sb, bk_d)
            vbias_sb = pers.tile([P, NCH], F32)
            nc.sync.dma_start(vbias_sb, vbias_d)
            nc.sync.dma_start(wvs, wv_d.rearrange("(t p) o -> p t o", p=P))
            nc.sync.dma_start(wos, wo_d.rearrange("(t p) o -> p t o", p=P))

            # ---- Q staging in quarter-chunks; ACT queue ----
            qtc = []
            for n in range(4):
                t = sb.tile([P, 8, 256], BF16, tag="qstage", bufs=4, name="qtc")
                nc.scalar.dma_start(
                    t, qt_d[:, n * 256 : (n + 1) * 256].rearrange("(t p) r -> p t r", p=P)
                )
                qtc.append(t)

            # ---- constants ----
            ones_f32 = pers.tile([P, 1], F32)
            nc.gpsimd.memset(ones_f32, 1.0)
            ones_bf = pers.tile([P, 1], BF16)
            nc.vector.tensor_copy(ones_bf, ones_f32)
            ones_r = pers.tile([1, P], F32R)
            nc.vector.tensor_copy(ones_r, ones_f32[0:1, :].to_broadcast([1, P]))

            # ---- persistent activations ----
            qT = pers.tile([P, 2, LQ], BF16)        # [ch%128, ch//128, q]
            kT = pers.tile([P, 2, LK], BF16)        # [ch%128, ch//128, key]
            v65 = pers.tile([P, NCH, NH, D + 1], BF16)  # [key%128, chunk, h, v|1]
            attnT = pers.tile([P, 2, LQ], BF16)
            e2T = pers.tile([P, NCH, LQ], BF16)     # head-2 exp store (64KB/part)

            # ones column of v65 (denominator trick)
            nc.vector.tensor_copy(
                v65[:, :, :, D].rearrange("p a b -> p (a b)"),
                ones_bf.to_broadcast([P, NCH * NH]),
            )

            # ---- Q projection (PSUM via pl tag: 4 x [128,512] tiles) ----
            for n in range(4):
                for mt in range(2):
                    pq = ps.tile([P, 512], F32, tag="pl", bufs=4, name="pq")
                    for i in range(8):
                        nc.tensor.matmul(
                            pq[:, 0:256], wqs[:, i, mt * P : (mt + 1) * P],
                            qtc[n][:, i, :],
                            start=(i == 0), stop=(i == 7),
                        )
                    nc.vector.tensor_scalar_add(
                        qT[:, mt, n * 256 : (n + 1) * 256], pq[:, 0:256],
                        bq_sb[:, mt : mt + 1],
                    )

            # ---- K & V projection pieces for one 512-key half; Pool DMA ----
            def half_loads(lo):
                ktq = sb.tile([P, 8, 512], BF16, tag="kstage", bufs=2, name="ktq")
                nc.gpsimd.dma_start(
                    ktq, kt_d[:, lo : lo + 512].rearrange("(t p) k -> p t k", p=P)
                )
                vtq = sb.tile([P, 8, 512], BF16, tag="vstage", bufs=2, name="vtq")
                nc.gpsimd.dma_start(
                    vtq, vt_d[:, lo : lo + 512].rearrange("(t p) k -> p t k", p=P)
                )
                return ktq, vtq

            def kproj(ktq, lo, mt):
                pk = ps.tile([P, 512], F32, tag="pl", bufs=4, name="pk")
                for i in range(8):
                    nc.tensor.matmul(
                        pk, wks[:, i, mt * P : (mt + 1) * P], ktq[:, i, :],
                        start=(i == 0), stop=(i == 7),
                    )
                nc.vector.tensor_scalar_add(
                    kT[:, mt, lo : lo + 512], pk, bk_sb[:, mt : mt + 1]
                )

            def vproj(vtq, c, a):
                pv = ps.tile([P, 512], F32, tag="pl", bufs=4, name="pv")
                for i in range(8):
                    nc.tensor.matmul(
                        pv[:, 0:CS], vtq[:, i, a * P : (a + 1) * P], wvs[:, i, :],
                        start=(i == 0), stop=(i == 7),
                    )
                nc.vector.tensor_copy(
                    v65[:, c, :, 0:D], pv[:, 0:CS].rearrange("p (h d) -> p h d", d=D)
                )

            # ---- attention pieces ----
            e_tiles = {}

            def logits_exp(h, c):
                ht, hp = h // 2, (h % 2) * D
                if h == 2:
                    dsts = (e2T[:, c, 0:512], e2T[:, c, 512:1024])
                else:
                    eT = sb.tile([P, LQ], BF16, tag="exp", bufs=5, name="eT")
                    dsts = (eT[:, 0:512], eT[:, 512:1024])
                    e_tiles[(h, c)] = eT
                for n in range(2):
                    pl = ps.tile([P, 512], F32, tag="pl", bufs=4, name="pl")
                    nc.tensor.matmul(
                        pl,
                        kT[hp : hp + D, ht, c * P : (c + 1) * P],
                        qT[hp : hp + D, ht, n * 512 : (n + 1) * 512],
                        start=True, stop=True, tile_position=(hp, 0),
                    )
                    nc.scalar.activation(
                        dsts[n], pl, AF.Exp, bias=vbias_sb[:, c : c + 1], scale=SCALE
                    )

            def av(h, c, po):
                src = e2T[:, c, :] if h == 2 else e_tiles.pop((h, c))
                for n in range(2):
                    nc.tensor.matmul(
                        po[0 : D + 1, n * 512 : (n + 1) * 512],
                        v65[:, c, h, :],
                        src[:, n * 512 : (n + 1) * 512],
                        start=(c == 0), stop=(c == NCH - 1),
                    )

            def make_finish(h, po):
                """Finish pieces for head h: rec (DVE), pb+bc (PE+DVE),
                tt (DVE). Interleave into the following instruction stream."""
                ht, hp = h // 2, (h % 2) * D
                state = {}

                def rec():
                    r = sb.tile([1, LQ], F32R, tag="rec", bufs=2, name="rec")
                    with nc.allow_low_precision(reason="softmax denom reciprocal"):
                        nc.vector.reciprocal(r, po[D : D + 1, :])
                    state["rec"] = r

                def pb_bc():
                    bc = sb.tile([D, LQ], F32, tag="bc", bufs=2, name="bc")
                    for n in range(2):
                        pb = ps.tile([P, 512], F32, tag="pl", bufs=4, name="pb")
                        nc.tensor.matmul(
                            pb[0:D, :], ones_r[:, 0:D],
                            state["rec"][:, n * 512 : (n + 1) * 512],
                            start=True, stop=True,
                        )
                        nc.vector.tensor_copy(bc[:, n * 512 : (n + 1) * 512], pb[0:D, :])
                    state["bc"] = bc

                def tt():
                    nc.vector.tensor_tensor(
                        attnT[hp : hp + D, ht, :], po[0:D, :], state["bc"], MUL
                    )

                return (rec, pb_bc, tt)

            # ---- phase 1: projections + heads 0,1 streamed + head 2 exps ----
            # Projection PSUM groups (pl-independent PE work for their 8-matmul
            # bodies) are spread between attention chunks so the PE never
            # outruns the 2-buffer pl rotation waiting on ACT; head-2 logits
            # lag one half-block behind to stay spread out.
            po0 = ps.tile([P, LQ], F32, tag="acc", bufs=2, name="po0")
            po1 = ps.tile([P, LQ], F32, tag="acc", bufs=2, name="po1")
            l2q = []
            loads = half_loads(0)
            for hb in range(2 * NQT):
                lo = hb * 512
                ktq, vtq = loads
                kproj(ktq, lo, 0)
                vproj(vtq, hb * 4 + 0, 0)
                for cc in range(4):
                    c = hb * 4 + cc
                    logits_exp(0, c)
                    if c > 0:
                        av(0, c - 1, po0)
                    logits_exp(1, c)
                    if c > 0:
                        av(1, c - 1, po1)
                    if cc == 1 and hb < 2 * NQT - 1:
                        loads = half_loads(lo + 512)
                    if cc < 3:
                        vproj(vtq, c + 1, cc + 1)
                    else:
                        kproj(ktq, lo, 1)
                    l2q.append(c)
                    while len(l2q) > 4:
                        logits_exp(2, l2q.pop(0))
            av(0, NCH - 1, po0)
            av(1, NCH - 1, po1)

            # ---- phase 2: head 3 live + head 2 AV replay; finishes ----
            fin0 = make_finish(0, po0)
            fin1 = make_finish(1, po1)
            po2 = ps.tile([P, LQ], F32, tag="acc", bufs=2, name="po2")
            po3 = ps.tile([P, LQ], F32, tag="acc", bufs=2, name="po3")
            AV2LAG, AV3LAG = 3, 4
            for c in range(NCH):
                logits_exp(3, c)
                if c == 0:
                    fin0[0]()
                elif c == 1:
                    fin0[1]()
                    fin1[0]()
                elif c == 2:
                    fin0[2]()
                    fin1[1]()
                elif c == 3:
                    fin1[2]()
                if l2q:
                    logits_exp(2, l2q.pop(0))
                if c >= AV2LAG:
                    av(2, c - AV2LAG, po2)
                if c >= AV3LAG:
                    av(3, c - AV3LAG, po3)
            for c in range(NCH - AV2LAG, NCH):
                av(2, c, po2)
            for c in range(NCH - AV3LAG, NCH):
                av(3, c, po3)
            fin2 = make_finish(2, po2)
            fin3 = make_finish(3, po3)

            # ---- output projection tail, finishes interleaved ----
            def outproj_halves(m, use_acc):
                if use_acc:
                    big = ps.tile([P, C], F32, tag="acc", bufs=2, name="pw")
                    return (big[:, 0:512], big[:, 512:1024])
                return (
                    ps.tile([P, 512], F32, tag="pl", bufs=4, name="pw"),
                    ps.tile([P, 512], F32, tag="pl", bufs=4, name="pw"),
                )

            def outproj_mm(pws, m, kc):
                for n in range(2):
                    nc.tensor.matmul(
                        pws[n],
                        attnT[:, kc, m * P : (m + 1) * P],
                        wos[:, kc, n * 512 : (n + 1) * 512],
                        start=(kc == 0), stop=(kc == 1),
                    )

            def outproj_store(pws, m):
                ob = sb.tile([P, C], F32, tag="ob", bufs=3, name="ob")
                nc.vector.tensor_copy(ob[:, 0:512], pws[0])
                nc.scalar.activation(ob[:, 512:1024], pws[1], AF.Copy)
                nc.sync.dma_start(out_d[m * P : (m + 1) * P, 0:512], ob[:, 0:512])
                nc.gpsimd.dma_start(
                    out_d[m * P : (m + 1) * P, 512:1024], ob[:, 512:1024]
                )

            # fin2/fin3 chains overlap each other; m=0's kc=0 accumulation
            # (needs only heads 0-1) overlaps fin3. Later m-tiles alternate
            # pl/acc pools (po2/po3 banks are free after tt).
            fin2[0]()
            fin2[1]()
            fin2[2]()
            pws0 = outproj_halves(0, use_acc=False)
            outproj_mm(pws0, 0, 0)
            fin3[0]()
            pws1 = outproj_halves(1, use_acc=True)  # po2's banks, free after tt2
            outproj_mm(pws1, 1, 0)
            fin3[1]()
            fin3[2]()
            outproj_mm(pws0, 0, 1)
            outproj_store(pws0, 0)
            outproj_mm(pws1, 1, 1)
            outproj_store(pws1, 1)
            for m in range(2, 8):
                pws = outproj_halves(m, use_acc=(m % 2 == 1))
                outproj_mm(pws, m, 0)
                outproj_mm(pws, m, 1)
                outproj_store(pws, m)

    nc.compile()
    return nc


_NC = None


def _get_nc():
    global _NC
    if _NC is None:
        _NC = build_nc()
    return _NC


def shard_inputs(Q, K_in, V_in, V_bias, Wq_w, Wq_b, Wk_w, Wk_b, Wv_w, Wv_b, Wo_w, Wo_b):
    """Build the 8 per-core input dicts (host transposes + bf16 casts)."""
    Q = np.asarray(Q)
    K_in = np.asarray(K_in)
    V_in = np.asarray(V_in)
    V_bias = np.asarray(V_bias)
    per_batch = []
    for b in range(2):
        per_batch.append({
            "qt": np.ascontiguousarray(Q[b].T).astype(NPBF16),
            "kt": np.ascontiguousarray(K_in[b].T).astype(NPBF16),
            "vt": np.ascontiguousarray(V_in[b].T).astype(NPBF16),
            "vbias": np.ascontiguousarray(V_bias[b].reshape(NCH, P).T),
        })
    in_maps = []
    for core in range(8):
        b, g = core // 4, core % 4
        gs, ge = g * CS, (g + 1) * CS
        in_maps.append({
            **per_batch[b],
            "wq": np.ascontiguousarray(Wq_w[gs:ge].T).astype(NPBF16),
            "wk": np.ascontiguousarray(Wk_w[gs:ge].T).astype(NPBF16),
            "wv": np.ascontiguousarray(Wv_w[gs:ge].T).astype(NPBF16),
            "wo": np.ascontiguousarray(Wo_w[:, gs:ge].T).astype(NPBF16),
            "bq": np.ascontiguousarray(Wq_b[gs:ge].reshape(2, P).T),
            "bk": np.ascontiguousarray(Wk_b[gs:ge].reshape(2, P).T),
        })
    return in_maps


def combine_outputs(results, Wv_b, Wo_w, Wo_b):
    """Sum the 4 head-group partials per batch; add output bias and the
    folded V-projection bias (attention weights sum to 1)."""
    bias = Wo_b + Wv_b @ Wo_w.T
    outs = np.stack([r["out"] for r in results]).reshape(2, 4, LQ, C)
    return (outs.sum(axis=1) + bias[None, None, :]).astype(np.float32)


def kernel(**inputs):
    nc = _get_nc()
    in_maps = shard_inputs(**inputs)
    res = bass_utils.run_bass_kernel_spmd(nc, in_maps, core_ids=list(range(8)))
    return combine_outputs(
        res.results,
        np.asarray(inputs["Wv_b"]),
        np.asarray(inputs["Wo_w"]),
        np.asarray(inputs["Wo_b"]),
    )


if __name__ == "__main__":
    rng = np.random.default_rng(0)
    ins = {
        "Q": rng.standard_normal((2, LQ, C), dtype=np.float32),
        "K_in": rng.standard_normal((2, LK, C), dtype=np.float32),
        "V_in": rng.standard_normal((2, LK, C), dtype=np.float32),
        "V_bias": rng.standard_normal((2, LK)).astype(np.float32),
        **{
            f"W{x}_w": (rng.standard_normal((C, C)) * 0.03).astype(np.float32)
            for x in "qkvo"
        },
        **{
            f"W{x}_b": (rng.standard_normal(C) * 0.03).astype(np.float32)
            for x in "qkvo"
        },
    }
    out = kernel(**ins)
    print("ok", out.shape, out.dtype)
